# revision 14
# baseline (speedup 1.0000x reference)
"""BitNet-style attention block (ternary-quantized QKV/proj) on 8 Trainium2 cores.

Strategy: data-parallel over batch (16 batches -> 2 per core, no collectives).
Everything on-chip runs in a feature-major ("transposed") layout:
  - x is staged host-side as x.T per core: [C, T] with T = 2048 tokens/core
  - QKV produces qkv.T = [d_out, tok]; Q/K slices spill to DRAM scratch,
    V is computed in natural [tok, d] layout straight into SBUF (augmented
    with a ones column so the attention row-sum l comes free out of the
    A@V matmul).
  - attention per (batch, head) streams key-blocks flash-style:
    St = K_blk.T-layout QK matmul -> exp on ACT (no max subtraction; logits
    are bounded ~|1| for this problem) -> accumulate (E, V|1) matmuls.
  - out.T accumulates in SBUF, proj emits y.T, host transposes back.
Matmuls run as float32r (full PE rate at free-dim >= 256, fp32 storage).
Ternary quantization (t in {-1,0,1}, w_q = t*s) happens on device; the
per-tensor scale s (a single scalar mean(|W|)) and threshold are computed
host-side in float64 for exact agreement with the reference's boundary
decisions, and folded in as w_q = t*s exactly.
"""

import os
import sys

import numpy as np

for _p in ("/opt/trn_rl_repo", "/root/.axon_site/_ro/trn_rl_repo"):
    if os.path.isdir(_p) and _p not in sys.path:
        sys.path.insert(0, _p)

import concourse.bass as bass
import concourse.mybir as mybir
import concourse.tile as tile
from concourse import bacc
from concourse.bass_utils import run_bass_kernel_spmd

B, N, C, H = 16, 1024, 768, 12
HD = C // H                    # 64
SCALE = float(HD ** -0.5)      # 0.125
EPS = 1e-5
NCORES = 8
BPC = B // NCORES              # 2 batches per core
T = BPC * N                    # 2048 tokens per core
P = 128
CB = C // P                    # 6 c-blocks
MQK = (2 * C) // P             # 12 m-blocks covering Q and K rows of qkv
TB = T // P                    # 16 token blocks
NQ = T // 512                  # 4 token chunks of 512
F32 = mybir.dt.float32
F32R = mybir.dt.float32r
AF = mybir.ActivationFunctionType
ALU = mybir.AluOpType

_CACHED_NC = None


def _split_drain_waits(nc):
    """The walrus build in this container accepts only one sync-wait per
    instruction; move extra waits onto preceding single-wait NoOps on the
    same engine (in-order queues make this semantics-preserving)."""
    for fn in nc.m.functions:
        for bb in fn.blocks:
            insts = bb.instructions
            i = 0
            while i < len(insts):
                inst = insts[i]
                si = getattr(inst, "sync_info", None)
                if (
                    si is not None
                    and si.on_wait is not None
                    and len(si.on_wait) > 1
                    # DMA waits are enforced at the DGE-queue level, not the
                    # sequencer; hoisting them onto a sequencer NoOp can
                    # deadlock (head-of-line blocking across queues).
                    and not type(inst).__name__.startswith("InstDMA")
                ):
                    waits = list(si.on_wait)
                    for j, w in enumerate(waits[:-1]):
                        nop = mybir.InstNoOp(
                            name=f"{inst.name}-prewait-{j}", ins=[], outs=[]
                        )
                        nop.engine = inst.engine
                        nop.sync_info = mybir.SyncInfo(on_wait=[w], on_update=[])
                        insts.insert(i, nop)
                        i += 1
                    inst.sync_info = mybir.SyncInfo(
                        on_wait=[waits[-1]], on_update=list(si.on_update)
                    )
                i += 1


def _build_nc(split=True):
    nc = bacc.Bacc(None)

    xT = nc.dram_tensor("xT", [C, T], F32, kind="ExternalInput")
    wqT = nc.dram_tensor("wqT", [C, 3 * C], F32, kind="ExternalInput")
    wpT = nc.dram_tensor("wpT", [C, C], F32, kind="ExternalInput")
    bp = nc.dram_tensor("bp", [C], F32, kind="ExternalInput")
    sq = nc.dram_tensor("sq", [1, 2], F32, kind="ExternalInput")  # [s, thr] qkv
    sp = nc.dram_tensor("sp", [1, 2], F32, kind="ExternalInput")  # [s, thr] proj
    cz = nc.dram_tensor("cz", [2, N], F32, kind="ExternalInput")  # row0=0.0, row1=1.0
    yT = nc.dram_tensor("yT", [C, T], F32, kind="ExternalOutput")

    with tile.TileContext(nc) as tc:
        with (
            tc.tile_pool(name="constp", bufs=1) as constp,
            tc.tile_pool(name="bigp", bufs=1) as bigp,
            tc.tile_pool(name="wqp", bufs=1) as wqp,
            tc.tile_pool(name="vaugp", bufs=1) as vaugp,
            tc.tile_pool(name="wvp", bufs=1) as wvp,
            tc.tile_pool(name="stagep", bufs=2) as stagep,
            tc.tile_pool(name="rawp", bufs=1) as rawp,
            tc.tile_pool(name="attnp", bufs=3) as attnp,
            tc.tile_pool(name="smallp", bufs=2) as smallp,
            tc.tile_pool(name="qkp", bufs=3) as qkp,
            tc.tile_pool(name="psp", bufs=6, space="PSUM") as psp,
            tc.tile_pool(name="avp", bufs=2, space="PSUM") as avp,
            tc.tile_pool(name="dramp", bufs=1, space="DRAM") as dramp,
            tc.tile_pool(name="dramls", bufs=3, space="DRAM") as dramls,
        ):
            # ---- DRAM scratch for transposed Q,K ----
            qk_d = dramp.tile([2 * C, T], F32, tag="qkd")

            # ---- load x.T ----
            x_sb = bigp.tile([P, CB, T], F32R, tag="big")
            nc.sync.dma_start(
                x_sb[:],
                xT[:, :].rearrange("(cb p) t -> p cb t", p=P).bitcast(F32R),
            )

            # ---- scalars: s / thr / -thr for both weight tensors ----
            sqb = constp.tile([P, 2], F32, tag="sqb")
            spb = constp.tile([P, 2], F32, tag="spb")
            nc.sync.dma_start(sqb[:], sq[:, :].to_broadcast([P, 2]))
            nc.sync.dma_start(spb[:], sp[:, :].to_broadcast([P, 2]))
            nthr_q = constp.tile([P, 1], F32, tag="nthr_q")
            nthr_p = constp.tile([P, 1], F32, tag="nthr_p")
            nc.vector.tensor_scalar_mul(nthr_q[:], sqb[:, 1:2], -1.0)
            nc.vector.tensor_scalar_mul(nthr_p[:], spb[:, 1:2], -1.0)

            # ---- bias ----
            b_sb = constp.tile([P, CB], F32, tag="b_sb")
            nc.sync.dma_start(b_sb[:], bp[:].rearrange("(cb p) -> p cb", p=P))

            # ---- quantize w_qkv.T (w_q = t*s, t in {-1,0,1}) ----
            # Q,K columns (0:2C) live in wq_q; V columns (2C:3C) in wv_q,
            # whose pool slot is later recycled for wp_q.
            wq_q = wqp.tile([P, CB, 2 * C], F32R, tag="wq")
            wv_q = wvp.tile([P, CB, C], F32R, tag="wv")
            MCH = 128
            for m0 in range(0, 3 * C, MCH):
                raw = rawp.tile([P, CB, MCH], F32, tag="wraw")
                nc.sync.dma_start(
                    raw[:],
                    wqT[:, m0 : m0 + MCH].rearrange("(cb p) m -> p cb m", p=P),
                )
                if m0 < 2 * C:
                    dst = wq_q[:, :, m0 : m0 + MCH]
                else:
                    dst = wv_q[:, :, m0 - 2 * C : m0 - 2 * C + MCH]
                # pos = (raw > thr) * s ; raw = (raw < -thr) * s ; dst = pos - raw
                pos = rawp.tile([P, CB, MCH], F32, tag="wpos")
                nc.vector.tensor_scalar(
                    pos[:], raw[:], sqb[:, 1:2], sqb[:, 0:1], ALU.is_gt, ALU.mult
                )
                nc.vector.tensor_scalar(
                    raw[:], raw[:], nthr_q[:], sqb[:, 0:1], ALU.is_lt, ALU.mult
                )
                nc.vector.tensor_sub(dst, pos[:], raw[:])

            # ---- V-augmented tile: [tok_blk, head, 64 vals + 1] ----
            v_aug = vaugp.tile([P, TB, H, HD + 1], F32R, tag="vaug")
            ones_col = constp.tile([P, 1], F32R, tag="ones_col")
            nc.sync.dma_start(
                ones_col[:], cz[1:2, 0:1].to_broadcast([P, 1]).bitcast(F32R)
            )
            nc.vector.tensor_copy(
                v_aug[:, :, :, HD : HD + 1],
                ones_col[:, None, :].to_broadcast([P, TB, H, 1]),
            )

            # ---- QKV: Q.T / K.T -> DRAM spill ----
            for mi in range(MQK):
                for qc in range(NQ):
                    ps = psp.tile([P, 512], F32, tag="ps")
                    for ci in range(CB):
                        nc.tensor.matmul(
                            ps[:],
                            wq_q[:, ci, mi * P : (mi + 1) * P],
                            x_sb[:, ci, qc * 512 : (qc + 1) * 512],
                            start=(ci == 0),
                            stop=(ci == CB - 1),
                        )
                    st = stagep.tile([P, 512], F32, tag="evac")
                    nc.vector.tensor_copy(st[:], ps[:])
                    nc.sync.dma_start(
                        qk_d[mi * P : (mi + 1) * P, qc * 512 : (qc + 1) * 512],
                        st[:],
                    )

            # ---- V natural layout into v_aug ----
            for tb in range(TB):
                for nch in range(2):
                    ps = psp.tile([P, 512], F32, tag="ps")
                    for ci in range(CB):
                        nc.tensor.matmul(
                            ps[:, :384],
                            x_sb[:, ci, tb * P : (tb + 1) * P],
                            wv_q[:, ci, nch * 384 : (nch + 1) * 384],
                            start=(ci == 0),
                            stop=(ci == CB - 1),
                        )
                    nc.vector.tensor_copy(
                        v_aug[:, tb, nch * 6 : (nch + 1) * 6, 0:HD],
                        ps[:, :384].rearrange("p (h d) -> p h d", d=HD),
                    )

            # ---- quantize w_proj.T (recycles the wv_q slot) ----
            wp_q = wvp.tile([P, CB, C], F32R, tag="wv")
            for m0 in range(0, C, MCH):
                raw = rawp.tile([P, CB, MCH], F32, tag="wraw")
                nc.sync.dma_start(
                    raw[:],
                    wpT[:, m0 : m0 + MCH].rearrange("(cb p) m -> p cb m", p=P),
                )
                dst = wp_q[:, :, m0 : m0 + MCH]
                pos = rawp.tile([P, CB, MCH], F32, tag="wpos")
                nc.vector.tensor_scalar(
                    pos[:], raw[:], spb[:, 1:2], spb[:, 0:1], ALU.is_gt, ALU.mult
                )
                nc.vector.tensor_scalar(
                    raw[:], raw[:], nthr_p[:], spb[:, 0:1], ALU.is_lt, ALU.mult
                )
                nc.vector.tensor_sub(dst, pos[:], raw[:])

            # ---- attention (out.T accumulates into the x_sb slot's pool) ----
            outT = bigp.tile([P, CB, T], F32R, tag="big")
            for b in range(BPC):
                for h in range(H):
                    qt = qkp.tile([P, N], F32R, tag="qt")
                    kt = qkp.tile([P, N], F32R, tag="qt")
                    nc.sync.dma_start(
                        qt[0:HD, :],
                        qk_d[h * HD : (h + 1) * HD, b * N : (b + 1) * N].bitcast(F32R),
                    )
                    nc.sync.dma_start(
                        qt[HD:P, :],
                        cz[0:1, :].to_broadcast([P - HD, N]).bitcast(F32R),
                    )
                    nc.sync.dma_start(
                        kt[0:HD, :],
                        qk_d[C + h * HD : C + (h + 1) * HD, b * N : (b + 1) * N].bitcast(F32R),
                    )
                    nc.sync.dma_start(
                        kt[HD:P, :],
                        cz[0:1, :].to_broadcast([P - HD, N]).bitcast(F32R),
                    )
                    for qc in range(2):
                        av = avp.tile([P, 512], F32, tag="av")
                        for kb in range(8):
                            st = psp.tile([P, 512], F32, tag="ps")
                            nc.tensor.matmul(
                                st[:],
                                kt[:, kb * P : (kb + 1) * P],
                                qt[:, qc * 512 : (qc + 1) * 512],
                                start=True,
                                stop=True,
                            )
                            e = attnp.tile([P, 512], F32R, tag="e")
                            nc.scalar.activation(
                                e[:], st[:], AF.Exp, bias=0.0, scale=SCALE
                            )
                            nc.tensor.matmul(
                                av[0 : HD + 1, :],
                                v_aug[:, b * 8 + kb, h, :],
                                e[:],
                                start=(kb == 0),
                                stop=(kb == 7),
                            )
                        linv = smallp.tile([1, 512], F32, tag="linv")
                        nc.vector.reciprocal(linv[:], av[HD : HD + 1, :])
                        ldram = dramls.tile([1, 512], F32, tag="ld")
                        nc.sync.dma_start(ldram[:], linv[:])
                        bc = smallp.tile([HD, 512], F32, tag="bc")
                        nc.sync.dma_start(bc[:], ldram[:, :].to_broadcast([HD, 512]))
                        nc.vector.tensor_mul(
                            out=outT[
                                (h % 2) * HD : (h % 2) * HD + HD,
                                h // 2,
                                b * N + qc * 512 : b * N + (qc + 1) * 512,
                            ],
                            in0=av[0:HD, :],
                            in1=bc[:],
                        )

            # ---- proj: y.T = wp_q.T-contract(out.T) + b ----
            for co in range(CB):
                for qc in range(NQ):
                    ps = psp.tile([P, 512], F32, tag="ps")
                    for ci in range(CB):
                        nc.tensor.matmul(
                            ps[:],
                            wp_q[:, ci, co * P : (co + 1) * P],
                            outT[:, ci, qc * 512 : (qc + 1) * 512],
                            start=(ci == 0),
                            stop=(ci == CB - 1),
                        )
                    yst = stagep.tile([P, 512], F32, tag="evac")
                    nc.scalar.activation(
                        yst[:], ps[:], AF.Identity, bias=b_sb[:, co : co + 1], scale=1.0
                    )
                    nc.sync.dma_start(
                        yT[co * P : (co + 1) * P, qc * 512 : (qc + 1) * 512], yst[:]
                    )

    # Bacc.finalize() -> compile() runs the canonical TRN2 legalization,
    # including generate_event_semaphores (splits waits to <=1 per
    # instruction, the constraint this walrus build enforces).
    nc.finalize()
    return nc


def _get_nc(split=True):
    global _CACHED_NC
    if _CACHED_NC is None:
        _CACHED_NC = _build_nc(split=split)
    return _CACHED_NC


def _scale_pair(w):
    s = np.float32(np.mean(np.abs(w), dtype=np.float64))
    thr = np.float32(0.5) * (s + np.float32(EPS))
    return np.array([[s, thr]], dtype=np.float32)


def run(x, w_qkv, w_proj, b_proj, trace=False):
    x = np.ascontiguousarray(x, dtype=np.float32)
    wqT = np.ascontiguousarray(np.asarray(w_qkv, dtype=np.float32).T)
    wpT = np.ascontiguousarray(np.asarray(w_proj, dtype=np.float32).T)
    bp = np.ascontiguousarray(b_proj, dtype=np.float32)
    sq = _scale_pair(w_qkv)
    sp = _scale_pair(w_proj)
    cz_host = np.zeros((2, N), dtype=np.float32)
    cz_host[1, :] = 1.0

    in_maps = []
    for c in range(NCORES):
        xs = x[c * BPC : (c + 1) * BPC].reshape(T, C)
        in_maps.append(
            {
                "xT": np.ascontiguousarray(xs.T),
                "wqT": wqT,
                "wpT": wpT,
                "bp": bp,
                "sq": sq,
                "sp": sp,
                "cz": cz_host,
            }
        )

    nc = _get_nc()
    res = run_bass_kernel_spmd(
        nc, in_maps, core_ids=list(range(NCORES)), trace=trace
    )

    y = np.empty((B, N, C), dtype=np.float32)
    for c in range(NCORES):
        yT_c = res.results[c]["yT"]  # [C, T]
        y[c * BPC : (c + 1) * BPC] = yT_c.T.reshape(BPC, N, C)
    return y, res


def kernel(x, w_qkv, w_proj, b_proj):
    y, _ = run(x, w_qkv, w_proj, b_proj, trace=False)
    return y


# revision 15
# speedup vs baseline: 1.0760x; 1.0760x over previous
"""BitNet-style attention block (ternary-quantized QKV/proj) on 8 Trainium2 cores.

Strategy: data-parallel over batch (16 batches -> 2 per core, no collectives).
Everything on-chip runs in a feature-major ("transposed") layout:
  - x is staged host-side as x.T per core: [C, T] with T = 2048 tokens/core
  - QKV produces qkv.T = [d_out, tok]; Q/K slices spill to DRAM scratch,
    V is computed in natural [tok, d] layout straight into SBUF (augmented
    with a ones column so the attention row-sum l comes free out of the
    A@V matmul).
  - attention per (batch, head) streams key-blocks flash-style:
    St = K_blk.T-layout QK matmul -> exp on ACT (no max subtraction; logits
    are bounded ~|1| for this problem) -> accumulate (E, V|1) matmuls.
  - out.T accumulates in SBUF, proj emits y.T, host transposes back.
Matmuls run as float32r (full PE rate at free-dim >= 256, fp32 storage).
Ternary quantization (t in {-1,0,1}, w_q = t*s) happens on device; the
per-tensor scale s (a single scalar mean(|W|)) and threshold are computed
host-side in float64 for exact agreement with the reference's boundary
decisions, and folded in as w_q = t*s exactly.
"""

import os
import sys

import ml_dtypes
import numpy as np

for _p in ("/opt/trn_rl_repo", "/root/.axon_site/_ro/trn_rl_repo"):
    if os.path.isdir(_p) and _p not in sys.path:
        sys.path.insert(0, _p)

import concourse.bass as bass
import concourse.mybir as mybir
import concourse.tile as tile
from concourse import bacc
from concourse.bass_utils import run_bass_kernel_spmd

B, N, C, H = 16, 1024, 768, 12
HD = C // H                    # 64
SCALE = float(HD ** -0.5)      # 0.125
EPS = 1e-5
NCORES = 8
BPC = B // NCORES              # 2 batches per core
T = BPC * N                    # 2048 tokens per core
P = 128
CB = C // P                    # 6 c-blocks
MQK = (2 * C) // P             # 12 m-blocks covering Q and K rows of qkv
TB = T // P                    # 16 token blocks
NQ = T // 512                  # 4 token chunks of 512
F32 = mybir.dt.float32
F32R = mybir.dt.float32r
BF16 = mybir.dt.bfloat16
AF = mybir.ActivationFunctionType
ALU = mybir.AluOpType

_CACHED_NC = None


def _split_drain_waits(nc):
    """The walrus build in this container accepts only one sync-wait per
    instruction; move extra waits onto preceding single-wait NoOps on the
    same engine (in-order queues make this semantics-preserving)."""
    for fn in nc.m.functions:
        for bb in fn.blocks:
            insts = bb.instructions
            i = 0
            while i < len(insts):
                inst = insts[i]
                si = getattr(inst, "sync_info", None)
                if (
                    si is not None
                    and si.on_wait is not None
                    and len(si.on_wait) > 1
                    # DMA waits are enforced at the DGE-queue level, not the
                    # sequencer; hoisting them onto a sequencer NoOp can
                    # deadlock (head-of-line blocking across queues).
                    and not type(inst).__name__.startswith("InstDMA")
                ):
                    waits = list(si.on_wait)
                    for j, w in enumerate(waits[:-1]):
                        nop = mybir.InstNoOp(
                            name=f"{inst.name}-prewait-{j}", ins=[], outs=[]
                        )
                        nop.engine = inst.engine
                        nop.sync_info = mybir.SyncInfo(on_wait=[w], on_update=[])
                        insts.insert(i, nop)
                        i += 1
                    inst.sync_info = mybir.SyncInfo(
                        on_wait=[waits[-1]], on_update=list(si.on_update)
                    )
                i += 1


def _build_nc(split=True):
    nc = bacc.Bacc(None)

    xT = nc.dram_tensor("xT", [C, T], BF16, kind="ExternalInput")
    wqT = nc.dram_tensor("wqT", [C, 3 * C], F32, kind="ExternalInput")
    wpT = nc.dram_tensor("wpT", [C, C], F32, kind="ExternalInput")
    bp = nc.dram_tensor("bp", [C], F32, kind="ExternalInput")
    sq = nc.dram_tensor("sq", [1, 2], F32, kind="ExternalInput")  # [s, thr] qkv
    sp = nc.dram_tensor("sp", [1, 2], F32, kind="ExternalInput")  # [s, thr] proj
    cz = nc.dram_tensor("cz", [2, N], BF16, kind="ExternalInput")  # row0=0.0, row1=1.0
    yT = nc.dram_tensor("yT", [C, T], F32, kind="ExternalOutput")

    with tile.TileContext(nc) as tc:
        with (
            tc.tile_pool(name="constp", bufs=1) as constp,
            tc.tile_pool(name="bigp", bufs=1) as bigp,
            tc.tile_pool(name="wqp", bufs=1) as wqp,
            tc.tile_pool(name="vaugp", bufs=1) as vaugp,
            tc.tile_pool(name="wvp", bufs=1) as wvp,
            tc.tile_pool(name="stagep", bufs=2) as stagep,
            tc.tile_pool(name="rawp", bufs=1) as rawp,
            tc.tile_pool(name="attnp", bufs=3) as attnp,
            tc.tile_pool(name="smallp", bufs=2) as smallp,
            tc.tile_pool(name="qkp", bufs=3) as qkp,
            tc.tile_pool(name="psp", bufs=6, space="PSUM") as psp,
            tc.tile_pool(name="avp", bufs=2, space="PSUM") as avp,
            tc.tile_pool(name="dramp", bufs=1, space="DRAM") as dramp,
            tc.tile_pool(name="dramls", bufs=3, space="DRAM") as dramls,
        ):
            # ---- DRAM scratch for transposed Q,K ----
            qk_d = dramp.tile([2 * C, T], BF16, tag="qkd")

            # ---- load x.T ----
            x_sb = bigp.tile([P, CB, T], BF16, tag="big")
            nc.sync.dma_start(
                x_sb[:], xT[:, :].rearrange("(cb p) t -> p cb t", p=P)
            )

            # ---- scalars: s / thr / -thr for both weight tensors ----
            sqb = constp.tile([P, 2], F32, tag="sqb")
            spb = constp.tile([P, 2], F32, tag="spb")
            nc.sync.dma_start(sqb[:], sq[:, :].to_broadcast([P, 2]))
            nc.sync.dma_start(spb[:], sp[:, :].to_broadcast([P, 2]))
            nthr_q = constp.tile([P, 1], F32, tag="nthr_q")
            nthr_p = constp.tile([P, 1], F32, tag="nthr_p")
            nc.vector.tensor_scalar_mul(nthr_q[:], sqb[:, 1:2], -1.0)
            nc.vector.tensor_scalar_mul(nthr_p[:], spb[:, 1:2], -1.0)

            # ---- bias ----
            b_sb = constp.tile([P, CB], F32, tag="b_sb")
            nc.sync.dma_start(b_sb[:], bp[:].rearrange("(cb p) -> p cb", p=P))

            # ---- quantize w_qkv.T (w_q = t*s, t in {-1,0,1}) ----
            # Q,K columns (0:2C) live in wq_q; V columns (2C:3C) in wv_q,
            # whose pool slot is later recycled for wp_q.
            wq_q = wqp.tile([P, CB, 2 * C], BF16, tag="wq")
            wv_q = wvp.tile([P, CB, C], BF16, tag="wv")
            MCH = 128
            for m0 in range(0, 3 * C, MCH):
                raw = rawp.tile([P, CB, MCH], F32, tag="wraw")
                nc.sync.dma_start(
                    raw[:],
                    wqT[:, m0 : m0 + MCH].rearrange("(cb p) m -> p cb m", p=P),
                )
                if m0 < 2 * C:
                    dst = wq_q[:, :, m0 : m0 + MCH]
                else:
                    dst = wv_q[:, :, m0 - 2 * C : m0 - 2 * C + MCH]
                # t = (raw > thr) - (raw < -thr)  in exact bf16 {-1,0,1};
                # the f32 scale s is applied at PSUM evacuation instead.
                pos = rawp.tile([P, CB, MCH], BF16, tag="wpos")
                neg = rawp.tile([P, CB, MCH], BF16, tag="wneg")
                nc.vector.tensor_scalar(
                    pos[:], raw[:], sqb[:, 1:2], None, ALU.is_gt
                )
                nc.vector.tensor_scalar(
                    neg[:], raw[:], nthr_q[:], None, ALU.is_lt
                )
                nc.vector.tensor_sub(dst, pos[:], neg[:])

            # ---- V-augmented tile: [tok_blk, head, 64 vals + 1] ----
            v_aug = vaugp.tile([P, TB, H, HD + 1], BF16, tag="vaug")
            ones_col = constp.tile([P, 1], BF16, tag="ones_col")
            nc.sync.dma_start(ones_col[:], cz[1:2, 0:1].to_broadcast([P, 1]))
            nc.vector.tensor_copy(
                v_aug[:, :, :, HD : HD + 1],
                ones_col[:, None, :].to_broadcast([P, TB, H, 1]),
            )

            # ---- QKV: Q.T / K.T -> DRAM spill ----
            for mi in range(MQK):
                for qc in range(NQ):
                    ps = psp.tile([P, 512], F32, tag="ps")
                    for ci in range(CB):
                        nc.tensor.matmul(
                            ps[:],
                            wq_q[:, ci, mi * P : (mi + 1) * P],
                            x_sb[:, ci, qc * 512 : (qc + 1) * 512],
                            start=(ci == 0),
                            stop=(ci == CB - 1),
                        )
                    st = stagep.tile([P, 512], BF16, tag="evac")
                    nc.vector.tensor_scalar_mul(st[:], ps[:], sqb[:, 0:1])
                    nc.sync.dma_start(
                        qk_d[mi * P : (mi + 1) * P, qc * 512 : (qc + 1) * 512],
                        st[:],
                    )

            # ---- V natural layout into v_aug ----
            for tb in range(TB):
                for nch in range(2):
                    ps = psp.tile([P, 512], F32, tag="ps")
                    for ci in range(CB):
                        nc.tensor.matmul(
                            ps[:, :384],
                            x_sb[:, ci, tb * P : (tb + 1) * P],
                            wv_q[:, ci, nch * 384 : (nch + 1) * 384],
                            start=(ci == 0),
                            stop=(ci == CB - 1),
                        )
                    nc.vector.tensor_scalar_mul(
                        v_aug[:, tb, nch * 6 : (nch + 1) * 6, 0:HD],
                        ps[:, :384].rearrange("p (h d) -> p h d", d=HD),
                        sqb[:, 0:1],
                    )

            # ---- quantize w_proj.T (recycles the wv_q slot) ----
            wp_q = wvp.tile([P, CB, C], BF16, tag="wv")
            for m0 in range(0, C, MCH):
                raw = rawp.tile([P, CB, MCH], F32, tag="wraw")
                nc.sync.dma_start(
                    raw[:],
                    wpT[:, m0 : m0 + MCH].rearrange("(cb p) m -> p cb m", p=P),
                )
                dst = wp_q[:, :, m0 : m0 + MCH]
                pos = rawp.tile([P, CB, MCH], BF16, tag="wpos")
                neg = rawp.tile([P, CB, MCH], BF16, tag="wneg")
                nc.vector.tensor_scalar(
                    pos[:], raw[:], spb[:, 1:2], None, ALU.is_gt
                )
                nc.vector.tensor_scalar(
                    neg[:], raw[:], nthr_p[:], None, ALU.is_lt
                )
                nc.vector.tensor_sub(dst, pos[:], neg[:])

            # ---- attention (out.T accumulates into the x_sb slot's pool) ----
            outT = bigp.tile([P, CB, T], BF16, tag="big")
            for b in range(BPC):
                for h in range(H):
                    qt = qkp.tile([P, N], BF16, tag="qt")
                    kt = qkp.tile([P, N], BF16, tag="qt")
                    nc.sync.dma_start(
                        qt[0:HD, :],
                        qk_d[h * HD : (h + 1) * HD, b * N : (b + 1) * N],
                    )
                    nc.sync.dma_start(
                        qt[HD:P, :],
                        cz[0:1, :].to_broadcast([P - HD, N]),
                    )
                    nc.sync.dma_start(
                        kt[0:HD, :],
                        qk_d[C + h * HD : C + (h + 1) * HD, b * N : (b + 1) * N],
                    )
                    nc.sync.dma_start(
                        kt[HD:P, :],
                        cz[0:1, :].to_broadcast([P - HD, N]),
                    )
                    for qc in range(2):
                        av = avp.tile([P, 512], F32, tag="av")
                        for kb in range(8):
                            st = psp.tile([P, 512], F32, tag="ps")
                            nc.tensor.matmul(
                                st[:],
                                kt[:, kb * P : (kb + 1) * P],
                                qt[:, qc * 512 : (qc + 1) * 512],
                                start=True,
                                stop=True,
                            )
                            e = attnp.tile([P, 512], BF16, tag="e")
                            nc.scalar.activation(
                                e[:], st[:], AF.Exp, bias=0.0, scale=SCALE
                            )
                            nc.tensor.matmul(
                                av[0 : HD + 1, :],
                                v_aug[:, b * 8 + kb, h, :],
                                e[:],
                                start=(kb == 0),
                                stop=(kb == 7),
                            )
                        linv = smallp.tile([1, 512], F32, tag="linv")
                        nc.vector.reciprocal(linv[:], av[HD : HD + 1, :])
                        ldram = dramls.tile([1, 512], F32, tag="ld")
                        nc.sync.dma_start(ldram[:], linv[:])
                        bc = smallp.tile([HD, 512], F32, tag="bc")
                        nc.sync.dma_start(bc[:], ldram[:, :].to_broadcast([HD, 512]))
                        nc.vector.tensor_mul(
                            out=outT[
                                (h % 2) * HD : (h % 2) * HD + HD,
                                h // 2,
                                b * N + qc * 512 : b * N + (qc + 1) * 512,
                            ],
                            in0=av[0:HD, :],
                            in1=bc[:],
                        )

            # ---- proj: y.T = wp_q.T-contract(out.T) + b ----
            for co in range(CB):
                for qc in range(NQ):
                    ps = psp.tile([P, 512], F32, tag="ps")
                    for ci in range(CB):
                        nc.tensor.matmul(
                            ps[:],
                            wp_q[:, ci, co * P : (co + 1) * P],
                            outT[:, ci, qc * 512 : (qc + 1) * 512],
                            start=(ci == 0),
                            stop=(ci == CB - 1),
                        )
                    yst = stagep.tile([P, 512], F32, tag="evac")
                    nc.scalar.activation(
                        yst[:],
                        ps[:],
                        AF.Identity,
                        bias=b_sb[:, co : co + 1],
                        scale=spb[:, 0:1],
                    )
                    nc.sync.dma_start(
                        yT[co * P : (co + 1) * P, qc * 512 : (qc + 1) * 512], yst[:]
                    )

    # Bacc.finalize() -> compile() runs the canonical TRN2 legalization,
    # including generate_event_semaphores (splits waits to <=1 per
    # instruction, the constraint this walrus build enforces).
    nc.finalize()
    return nc


def _get_nc(split=True):
    global _CACHED_NC
    if _CACHED_NC is None:
        _CACHED_NC = _build_nc(split=split)
    return _CACHED_NC


def _scale_pair(w):
    s = np.float32(np.mean(np.abs(w), dtype=np.float64))
    thr = np.float32(0.5) * (s + np.float32(EPS))
    return np.array([[s, thr]], dtype=np.float32)


def run(x, w_qkv, w_proj, b_proj, trace=False):
    x = np.ascontiguousarray(x, dtype=np.float32)
    wqT = np.ascontiguousarray(np.asarray(w_qkv, dtype=np.float32).T)
    wpT = np.ascontiguousarray(np.asarray(w_proj, dtype=np.float32).T)
    bp = np.ascontiguousarray(b_proj, dtype=np.float32)
    sq = _scale_pair(w_qkv)
    sp = _scale_pair(w_proj)
    cz_host = np.zeros((2, N), dtype=ml_dtypes.bfloat16)
    cz_host[1, :] = 1.0

    in_maps = []
    for c in range(NCORES):
        xs = x[c * BPC : (c + 1) * BPC].reshape(T, C)
        in_maps.append(
            {
                "xT": np.ascontiguousarray(xs.T).astype(ml_dtypes.bfloat16),
                "wqT": wqT,
                "wpT": wpT,
                "bp": bp,
                "sq": sq,
                "sp": sp,
                "cz": cz_host,
            }
        )

    nc = _get_nc()
    res = run_bass_kernel_spmd(
        nc, in_maps, core_ids=list(range(NCORES)), trace=trace
    )

    y = np.empty((B, N, C), dtype=np.float32)
    for c in range(NCORES):
        yT_c = res.results[c]["yT"]  # [C, T]
        y[c * BPC : (c + 1) * BPC] = yT_c.T.reshape(BPC, N, C)
    return y, res


def kernel(x, w_qkv, w_proj, b_proj):
    y, _ = run(x, w_qkv, w_proj, b_proj, trace=False)
    return y
